# revision 6
# baseline (speedup 1.0000x reference)
"""Trainium2 Bass kernel for the shared-weight multi-head attention problem.

Math (per batch b, head h, with x_h = x[b,:,h*64:(h+1)*64] [S, d]):
    q = k = x_h @ W + b          (b is zero in this problem, handled anyway)
    s = q @ q^T / d              (symmetric!)
    t = s + (1-mask_q)*(-1e6)    (constant per softmax ROW -> softmax-invariant;
                                  only effect in the reference is fp32 score
                                  quantization on masked rows, ~8e-3 global rel,
                                  well inside the 2e-2 gate -> we drop it)
    out_h = softmax(t) @ x_h

Device strategy (8 cores, data parallel over (batch, head-group-of-8)):
  - F orientation: score tiles [k partitions, q free]; k==q so F is symmetric.
  - Only the block upper triangle (512-wedge granularity) is computed by
    PE matmul + ACT exp; the mirrored lower-triangle tiles are produced by
    SBUF->SBUF DMA-transposes (XBAR), which run on otherwise-idle DMA engines.
    This nearly halves the ACT exp chain, the previous bottleneck.
  - Projection: qT = W^T @ xt (+b) with xt obtained by DMA-transposing the
    bf16 cast of x (two heads per 128-wide transpose).
  - PV: po[65,512] accumulates [x_h | 1]^T @ F over 16 k-tiles; row 64 gives
    the softmax denominators L (ones-column trick). PE transpose brings the
    output to natural [q, d] layout; DVE reciprocal+scale normalizes.
  - No max-subtraction (scores/64 are in [-1, 1]); exact to fp rounding.
"""

import numpy as np

B, S, D, H, d = 4, 2048, 1024, 16, 64
NH = 8          # heads per core
NCORES = 8

_NC_CACHE = {}


def _build_nc(s=S, nh=NH, reps=1):
    import concourse.bacc as bacc
    import concourse.tile as tile
    from concourse import mybir
    from concourse.masks import make_identity

    f32 = mybir.dt.float32
    bf16 = mybir.dt.bfloat16
    Exp = mybir.ActivationFunctionType.Exp

    kt = s // 128          # k tiles per head
    nj = s // 512          # q column blocks per head
    npair = nh // 2

    nc = bacc.Bacc("TRN2", target_bir_lowering=False, debug=False)

    x_in = nc.declare_dram_parameter("x", [s, nh * d], f32, isOutput=False)
    w_in = nc.declare_dram_parameter("W", [d, d], f32, isOutput=False)
    b_in = nc.declare_dram_parameter("b", [d, 1], f32, isOutput=False)
    out = nc.declare_dram_parameter("out", [nh, s, d], f32, isOutput=True)

    with tile.TileContext(nc) as tc:
        with (
            tc.tile_pool(name="const", bufs=1) as const_pool,
            tc.tile_pool(name="xf", bufs=3) as xf_pool,
            tc.tile_pool(name="xb", bufs=3) as xb_pool,
            tc.tile_pool(name="xh", bufs=1) as xh_pool,
            tc.tile_pool(name="xtb", bufs=1) as xtb_pool,
            tc.tile_pool(name="q", bufs=2) as q_pool,
            tc.tile_pool(name="F", bufs=2) as F_pool,
            tc.tile_pool(name="at", bufs=2) as at_pool,
            tc.tile_pool(name="ob", bufs=2) as ob_pool,
            tc.tile_pool(name="rl", bufs=2) as rl_pool,
            tc.tile_pool(name="ps", bufs=2, space="PSUM") as ps_pool,
            tc.tile_pool(name="po", bufs=2, space="PSUM") as po_pool,
            tc.tile_pool(name="pq", bufs=1, space="PSUM") as pq_pool,
            tc.tile_pool(name="pn", bufs=1, space="PSUM") as pn_pool,
        ):
            ident = const_pool.tile([128, 128], f32, tag="ident")
            make_identity(nc, ident[:])
            w_raw = const_pool.tile([128, d], f32, tag="wraw")
            nc.sync.dma_start(w_raw[0:d, :], w_in[:, :])
            nc.sync.dma_start(w_raw[d : 2 * d, :], w_in[:, :])
            # W replicated on both partition halves so lhsT can share the
            # rhs base partition (matmul requirement)
            w_sb = const_pool.tile([128, d], bf16, tag="w")
            nc.vector.tensor_copy(w_sb[:], w_raw[:])
            b_sb = const_pool.tile([d, 1], f32, tag="b")
            nc.sync.dma_start(b_sb[:], b_in[:, :])

            def emit_load():
                """Load x, cast to bf16: xh_all ([x_h|1] interleaved, PV
                stationary) and xtb (DMA-transposed, per head-pair)."""
                xh_all = xh_pool.tile([128, nh * kt * 65], bf16, tag="xh")
                xtb = [
                    xtb_pool.tile([128, s], bf16, tag=f"xtb{p}", name=f"xtb{p}")
                    for p in range(npair)
                ]
                xhv = xh_all[:].rearrange("p (h t c) -> p h t c", h=nh, t=kt)
                for t in range(kt):
                    xf = xf_pool.tile([128, nh * d], f32, tag="xf")
                    nc.sync.dma_start(xf[:], x_in[t * 128 : (t + 1) * 128, :])
                    xbt = xb_pool.tile([128, nh * d], bf16, tag="xb")
                    nc.vector.tensor_copy(xbt[:], xf[:])
                    # spread into per-head 65-col slots (col 64 = ones, below)
                    nc.vector.tensor_copy(
                        xhv[:, :, t, 0:64],
                        xbt[:].rearrange("p (h c) -> p h c", h=nh),
                    )
                    for p in range(npair):
                        nc.sync.dma_start(
                            xtb[p][:, t * 128 : (t + 1) * 128],
                            xbt[:, p * 128 : (p + 1) * 128],
                            transpose=True,
                        )
                nc.vector.memset(xhv[:, :, :, 64], 1.0)
                return xh_all, xtb

            def emit_proj(h, xtb):
                """qT[d, s] = W^T @ x_h^T + b, in bf16."""
                p, half = h // 2, h % 2
                qT = q_pool.tile([d, s], bf16, tag="qT")
                for j in range(nj):
                    pq = pq_pool.tile([d, 512], f32, tag="pq")
                    nc.tensor.matmul(
                        pq[:],
                        w_sb[half * d : (half + 1) * d, :],
                        xtb[p][half * d : (half + 1) * d, j * 512 : (j + 1) * 512],
                        start=True,
                        stop=True,
                    )
                    nc.vector.tensor_scalar_add(
                        qT[:, j * 512 : (j + 1) * 512], pq[:], b_sb[:]
                    )
                return qT

            def emit_scores_block(h, Jq, qT, Fv):
                """Score tiles (k-tiles 0..4Jq+3) x (q block Jq): matmul,
                exp into F, and DMA-transpose mirrors whose source is now
                complete (dest k-tile in this Jq's wedge)."""
                for ii in range(2 * Jq + 2):
                    ps = ps_pool.tile([128, 1024], f32, tag="ps")
                    for u in range(2):
                        I = 2 * ii + u
                        nc.tensor.matmul(
                            ps[:, u * 512 : (u + 1) * 512],
                            qT[:, I * 128 : (I + 1) * 128],
                            qT[:, Jq * 512 : (Jq + 1) * 512],
                            start=True,
                            stop=True,
                        )
                    nc.scalar.activation(
                        Fv[:, 2 * ii : 2 * ii + 2, Jq * 512 : (Jq + 1) * 512],
                        ps[:].rearrange("p (u c) -> p u c", u=2),
                        Exp,
                        bias=0.0,
                        scale=1.0 / 64.0,
                    )
                    # mirrors: dest (dI in this wedge, dJ < 4Jq), src (dJ, dI)
                    for u in range(2):
                        dJ = 2 * ii + u
                        if dJ < 4 * Jq:
                            for dI in range(4 * Jq, 4 * Jq + 4):
                                nc.sync.dma_start(
                                    Fv[:, dI, dJ * 128 : (dJ + 1) * 128],
                                    Fv[:, dJ, dI * 128 : (dI + 1) * 128],
                                    transpose=True,
                                )

            def emit_pv_block(h, Jq, xh_all, Fv):
                po = po_pool.tile([d + 1, 512], f32, tag="po")
                for t in range(kt):
                    c0 = (h * kt + t) * 65
                    nc.tensor.matmul(
                        po[:],
                        xh_all[:, c0 : c0 + 65],
                        Fv[:, t, Jq * 512 : (Jq + 1) * 512],
                        start=(t == 0),
                        stop=(t == kt - 1),
                    )
                at = at_pool.tile([d + 1, 512], f32, tag="at")
                nc.vector.tensor_copy(at[:], po[:])
                pn = pn_pool.tile([128, 4 * 65], f32, tag="pn")
                for u in range(4):
                    nc.tensor.transpose(
                        pn[:, u * 65 : (u + 1) * 65],
                        at[:, u * 128 : (u + 1) * 128],
                        ident[0:65, 0:65],
                    )
                pnv = pn[:].rearrange("p (u c) -> p u c", u=4)
                rl = rl_pool.tile([128, 4], f32, tag="rl")
                nc.vector.reciprocal(rl[:], pnv[:, :, 64])
                ob = ob_pool.tile([128, 4 * 64], f32, tag="ob")
                for u in range(4):
                    nc.vector.tensor_scalar_mul(
                        ob[:, u * 64 : (u + 1) * 64],
                        pnv[:, u, 0:64],
                        rl[:, u : u + 1],
                    )
                nc.sync.dma_start(
                    out[h, Jq * 512 : (Jq + 1) * 512, :].rearrange(
                        "(u r) e -> r u e", u=4
                    ),
                    ob[:].rearrange("p (u e) -> p u e", u=4),
                )

            for _rep in range(reps):
                xh_all, xtb = emit_load()
                qT = emit_proj(0, xtb)
                prev = None  # (h, Fv) with all mirrors emitted
                for h in range(nh):
                    F_t = F_pool.tile([128, kt * s], bf16, tag="F")
                    Fv = F_t[:].rearrange("p (t c) -> p t c", t=kt)
                    qT_next = None
                    for Jq in range(nj):
                        emit_scores_block(h, Jq, qT, Fv)
                        if Jq == 0 and h + 1 < nh:
                            qT_next = emit_proj(h + 1, xtb)
                        if prev is not None:
                            emit_pv_block(prev[0], Jq, xh_all, prev[1])
                    prev = (h, Fv)
                    if qT_next is not None:
                        qT = qT_next
                for Jq in range(nj):
                    emit_pv_block(prev[0], Jq, xh_all, prev[1])

    nc.compile()
    return nc


def get_nc(s=S, nh=NH, reps=1):
    key = (s, nh, reps)
    if key not in _NC_CACHE:
        _NC_CACHE[key] = _build_nc(s, nh, reps)
    return _NC_CACHE[key]


def make_in_maps(x, mask, W, b, s=S, nh=NH):
    """Shard full inputs into per-core input maps (core = batch*2 + head_group).

    mask is unused on-device: it only shifts whole softmax rows by a constant
    (softmax-invariant; the reference's fp32 quantization side effect is within
    the error budget)."""
    x = np.asarray(x, dtype=np.float32)
    W = np.ascontiguousarray(np.asarray(W, dtype=np.float32))
    bv = np.ascontiguousarray(np.asarray(b, dtype=np.float32).reshape(d, 1))
    in_maps = []
    for c in range(NCORES):
        bb, hg = c // 2, c % 2
        xs = np.ascontiguousarray(x[bb][:, hg * nh * d : (hg + 1) * nh * d])
        in_maps.append({"x": xs, "W": W, "b": bv})
    return in_maps


def gather_out(results):
    """results: list of 8 dicts with 'out' [NH, S, d] -> full [B, S, D]."""
    a = np.empty((B, H, S, d), np.float32)
    for c in range(NCORES):
        bb, hg = c // 2, c % 2
        a[bb, hg * NH : (hg + 1) * NH] = results[c]["out"]
    return a.reshape(B, S, D)


def kernel(x, mask, W, b):
    from concourse.bass_utils import run_bass_kernel_spmd

    nc = get_nc()
    in_maps = make_in_maps(x, mask, W, b)
    res = run_bass_kernel_spmd(nc, in_maps, list(range(NCORES)))
    return gather_out(res.results)


# revision 7
# speedup vs baseline: 2.9813x; 2.9813x over previous
"""Trainium2 Bass kernel for the shared-weight multi-head attention problem.

Math (per batch b, head h, with x_h = x[b,:,h*64:(h+1)*64] [S, d]):
    q = k = x_h @ W + b          (b is zero in this problem, handled anyway)
    s = q @ q^T / d              (symmetric)
    t = s + (1-mask_q)*(-1e6)    (constant per softmax ROW -> softmax-invariant;
                                  only effect in the reference is fp32 score
                                  quantization on masked rows, ~8e-3 global rel,
                                  well inside the 2e-2 gate -> dropped on device)
    out_h = softmax(t) @ x_h

Device strategy (8 cores, data parallel over (batch, head-group-of-8)):
  - F orientation: score tiles [k partitions, q free].
  - The ACT exp chain is the bottleneck: (N + 352)/1.2 ns per activation
    instruction. Scores are produced in [128, 1536] PSUM groups (3 matmuls)
    so each exp instruction amortizes the fixed 352-cycle access cost over
    1536 columns; k-tile-contiguous F layout keeps every exp output 2D.
  - PV for block (h,Jq) is software-pipelined (lag 1) and its 16 matmuls are
    interleaved between the next block's score matmul groups so the PE has
    work while ACT drains a group (ps bufs=2 of 3 PSUM banks each).
  - Projection: qT = W^T @ xt (+b); xt is built by XBAR DMA-transposes of the
    bf16 x copy (two heads per 128-wide transpose), spread just-in-time
    across the two HWDGE queues (sync + scalar) to stay off the engines.
  - PV: po[65,512] accumulates [x_h | 1]^T @ F over 16 k-tiles; row 64 gives
    softmax denominators L (ones-column trick). PE transpose brings output to
    natural [q, d]; DVE reciprocal+scale normalizes; one DMA per (h,Jq).
"""

import numpy as np

B, S, D, H, d = 4, 2048, 1024, 16, 64
NH = 8          # heads per core
NCORES = 8

# k-tile grouping for the scores->exp pipeline: 3-tile PSUM groups (3 banks)
GROUPS = [(0, 3), (3, 6), (6, 9), (9, 12), (12, 15), (15, 16)]
# how many PV matmuls of the lagged block to interleave after each group
PV_SPLIT = [3, 3, 3, 3, 2, 2]

_NC_CACHE = {}


def _build_nc(s=S, nh=NH, reps=1):
    import concourse.bacc as bacc
    import concourse.tile as tile
    from concourse import mybir
    from concourse.masks import make_identity

    f32 = mybir.dt.float32
    bf16 = mybir.dt.bfloat16
    Exp = mybir.ActivationFunctionType.Exp

    kt = s // 128          # 16 k tiles per head
    nj = s // 512          # 4 q column blocks per head
    npair = nh // 2

    nc = bacc.Bacc("TRN2", target_bir_lowering=False, debug=False)

    x_in = nc.declare_dram_parameter("x", [s, nh * d], f32, isOutput=False)
    w_in = nc.declare_dram_parameter("W", [d, d], f32, isOutput=False)
    b_in = nc.declare_dram_parameter("b", [d, 1], f32, isOutput=False)
    out = nc.declare_dram_parameter("out", [nh, s, d], f32, isOutput=True)

    with tile.TileContext(nc) as tc:
        with (
            tc.tile_pool(name="const", bufs=1) as const_pool,
            tc.tile_pool(name="xf", bufs=3) as xf_pool,
            tc.tile_pool(name="xb", bufs=1) as xb_pool,
            tc.tile_pool(name="xh", bufs=1) as xh_pool,
            tc.tile_pool(name="xtb", bufs=1) as xtb_pool,
            tc.tile_pool(name="q", bufs=2) as q_pool,
            tc.tile_pool(name="F", bufs=3) as F_pool,
            tc.tile_pool(name="at", bufs=2) as at_pool,
            tc.tile_pool(name="ob", bufs=2) as ob_pool,
            tc.tile_pool(name="rl", bufs=2) as rl_pool,
            tc.tile_pool(name="ps", bufs=2, space="PSUM") as ps_pool,
            tc.tile_pool(name="po", bufs=1, space="PSUM") as po_pool,
            tc.tile_pool(name="pn", bufs=1, space="PSUM") as pn_pool,
        ):
            ident = const_pool.tile([128, 128], f32, tag="ident")
            make_identity(nc, ident[:])
            w_raw = const_pool.tile([128, d], f32, tag="wraw")
            nc.sync.dma_start(w_raw[0:d, :], w_in[:, :])
            nc.sync.dma_start(w_raw[d : 2 * d, :], w_in[:, :])
            # W replicated on both partition halves so lhsT can share the
            # rhs base partition (matmul requirement)
            w_sb = const_pool.tile([128, d], bf16, tag="w")
            nc.vector.tensor_copy(w_sb[:], w_raw[:])
            b_sb = const_pool.tile([d, 1], f32, tag="b")
            nc.sync.dma_start(b_sb[:], b_in[:, :])

            dma_engines = [nc.sync, nc.scalar]
            dma_rr = [0]

            def dma(out_ap, in_ap, **kw):
                eng = dma_engines[dma_rr[0] % 2]
                dma_rr[0] += 1
                eng.dma_start(out_ap, in_ap, **kw)

            def emit_load():
                """Load x, cast to bf16: xh_all ([x_h|1] interleaved, PV
                stationary) and xball (contiguous, transpose source)."""
                xh_all = xh_pool.tile([128, nh * kt * 65], bf16, tag="xh")
                xball = xb_pool.tile([128, kt * nh * d], bf16, tag="xball")
                xhv = xh_all[:].rearrange("p (h t c) -> p h t c", h=nh, t=kt)
                for t in range(kt):
                    xf = xf_pool.tile([128, nh * d], f32, tag="xf")
                    dma(xf[:], x_in[t * 128 : (t + 1) * 128, :])
                    xbt = xball[:, t * 512 : (t + 1) * 512]
                    nc.vector.tensor_copy(xbt, xf[:])
                    nc.vector.tensor_copy(
                        xhv[:, :, t, 0:64],
                        xbt.rearrange("p (h c) -> p h c", h=nh),
                    )
                nc.vector.memset(xhv[:, :, :, 64], 1.0)
                return xh_all, xball

            def emit_xtb_pair(p, xball):
                """DMA-transpose pair p's x columns -> xtb [128, s] bf16
                (partitions 0:64 = head 2p, 64:128 = head 2p+1)."""
                xtb = xtb_pool.tile([128, s], bf16, tag=f"xtb{p}", name=f"xtb{p}")
                for t in range(kt):
                    dma(
                        xtb[:, t * 128 : (t + 1) * 128],
                        xball[:, t * 512 + p * 128 : t * 512 + (p + 1) * 128],
                        transpose=True,
                    )
                return xtb

            def emit_proj(h, xtb_p):
                """qT[d, s] = W^T @ x_h^T + b, in bf16."""
                half = h % 2
                qT = q_pool.tile([d, s], bf16, tag="qT")
                for j in range(nj):
                    pq = ps_pool.tile([d, 512], f32, tag="ps")
                    nc.tensor.matmul(
                        pq[:],
                        w_sb[half * d : (half + 1) * d, :],
                        xtb_p[half * d : (half + 1) * d, j * 512 : (j + 1) * 512],
                        start=True,
                        stop=True,
                    )
                    nc.vector.tensor_scalar_add(
                        qT[:, j * 512 : (j + 1) * 512], pq[:], b_sb[:]
                    )
                return qT

            def emit_score_group(Jq, g, qT, Fj):
                a, bb = GROUPS[g]
                w = (bb - a) * 512
                ps = ps_pool.tile([128, w], f32, tag="ps")
                for I in range(a, bb):
                    nc.tensor.matmul(
                        ps[:, (I - a) * 512 : (I - a + 1) * 512],
                        qT[:, I * 128 : (I + 1) * 128],
                        qT[:, Jq * 512 : (Jq + 1) * 512],
                        start=True,
                        stop=True,
                    )
                nc.scalar.activation(
                    Fj[:, a * 512 : bb * 512], ps[:], Exp,
                    bias=0.0, scale=1.0 / 64.0,
                )

            def emit_pv_chunk(h, Jq, t0, t1, po, xh_all, Fj):
                for t in range(t0, t1):
                    c0 = (h * kt + t) * 65
                    nc.tensor.matmul(
                        po[:],
                        xh_all[:, c0 : c0 + 65],
                        Fj[:, t * 512 : (t + 1) * 512],
                        start=(t == 0),
                        stop=(t == kt - 1),
                    )

            def emit_out(h, Jq, po):
                at = at_pool.tile([d + 1, 512], f32, tag="at")
                nc.vector.tensor_copy(at[:], po[:])
                pn = pn_pool.tile([128, 4 * 65], f32, tag="pn")
                for u in range(4):
                    nc.tensor.transpose(
                        pn[:, u * 65 : (u + 1) * 65],
                        at[:, u * 128 : (u + 1) * 128],
                        ident[0 : d + 1, 0 : d + 1],
                    )
                pnv = pn[:].rearrange("p (u c) -> p u c", u=4)
                rl = rl_pool.tile([128, 4], f32, tag="rl")
                nc.vector.reciprocal(rl[:], pnv[:, :, 64])
                ob = ob_pool.tile([128, 4 * 64], f32, tag="ob")
                for u in range(4):
                    nc.vector.tensor_scalar_mul(
                        ob[:, u * 64 : (u + 1) * 64],
                        pnv[:, u, 0:64],
                        rl[:, u : u + 1],
                    )
                dma(
                    out[h, Jq * 512 : (Jq + 1) * 512, :].rearrange(
                        "(u r) e -> r u e", u=4
                    ),
                    ob[:].rearrange("p (u e) -> p u e", u=4),
                )

            for _rep in range(reps):
                xh_all, xball = emit_load()
                xtb_p = emit_xtb_pair(0, xball)
                qT = emit_proj(0, xtb_p)
                pend = None  # (h, Jq, Fj) with scores emitted, PV not yet
                po = None
                for h in range(nh):
                    for Jq in range(nj):
                        Fj = F_pool.tile([128, kt * 512], bf16, tag="F")
                        if pend is not None:
                            po = po_pool.tile([d + 1, 512], f32, tag="po")
                        t0 = 0
                        for g in range(len(GROUPS)):
                            emit_score_group(Jq, g, qT, Fj)
                            if pend is not None:
                                t1 = t0 + PV_SPLIT[g]
                                emit_pv_chunk(
                                    pend[0], pend[1], t0, t1, po, xh_all, pend[2]
                                )
                                t0 = t1
                        if pend is not None:
                            emit_out(pend[0], pend[1], po)
                        pend = (h, Jq, Fj)
                        # prep work for upcoming heads, off the critical path
                        if Jq == 1 and h % 2 == 0 and h + 2 < nh:
                            xtb_next = emit_xtb_pair(h // 2 + 1, xball)
                        if Jq == 2 and h + 1 < nh:
                            qT_next = emit_proj(h + 1, xtb_p if h % 2 == 0 else xtb_next)
                    if h + 1 < nh:
                        qT = qT_next
                        if h % 2 == 1:
                            xtb_p = xtb_next
                po = po_pool.tile([d + 1, 512], f32, tag="po")
                emit_pv_chunk(pend[0], pend[1], 0, kt, po, xh_all, pend[2])
                emit_out(pend[0], pend[1], po)

    nc.compile()
    return nc


def get_nc(s=S, nh=NH, reps=1):
    key = (s, nh, reps)
    if key not in _NC_CACHE:
        _NC_CACHE[key] = _build_nc(s, nh, reps)
    return _NC_CACHE[key]


def make_in_maps(x, mask, W, b, s=S, nh=NH):
    """Shard full inputs into per-core input maps (core = batch*2 + head_group).

    mask is unused on-device: it only shifts whole softmax rows by a constant
    (softmax-invariant; the reference's fp32 quantization side effect is within
    the error budget)."""
    x = np.asarray(x, dtype=np.float32)
    W = np.ascontiguousarray(np.asarray(W, dtype=np.float32))
    bv = np.ascontiguousarray(np.asarray(b, dtype=np.float32).reshape(d, 1))
    in_maps = []
    for c in range(NCORES):
        bb, hg = c // 2, c % 2
        xs = np.ascontiguousarray(x[bb][:, hg * nh * d : (hg + 1) * nh * d])
        in_maps.append({"x": xs, "W": W, "b": bv})
    return in_maps


def gather_out(results):
    """results: list of 8 dicts with 'out' [NH, S, d] -> full [B, S, D]."""
    a = np.empty((B, H, S, d), np.float32)
    for c in range(NCORES):
        bb, hg = c // 2, c % 2
        a[bb, hg * NH : (hg + 1) * NH] = results[c]["out"]
    return a.reshape(B, S, D)


def kernel(x, mask, W, b):
    from concourse.bass_utils import run_bass_kernel_spmd

    nc = get_nc()
    in_maps = make_in_maps(x, mask, W, b)
    res = run_bass_kernel_spmd(nc, in_maps, list(range(NCORES)))
    return gather_out(res.results)
